# revision 34
# baseline (speedup 1.0000x reference)
"""Trainium2 Bass kernel for DCN (deformable conv v1) + GroupNorm + ReLU.

Problem: x[2,256,128,128], offset[2,18,128,128], weight[256,256,3,3],
bias/gamma/beta[256] -> relu(groupnorm(deform_conv(x, offset, weight) + bias)).

Sharding: 8 cores = 2 images x 4 row-bands of 32 output rows.
Per core pipeline (all on device):
  - offsets -> positions -> clamp -> int/frac (DVE+ACT); int16 token
    indices for dma_gather; bilinear weights stored DUPLICATED in pairs
    so broadcast APs keep a packed last dim (DVE 2x mode).
  - per (tap, 1024-px block): one dma_gather (elem = 4 bilinear corners
    x 256 ch bf16).  Bilinear fold via the ratio trick:
        sample = s00*(A + r*C) + s01*(B + r*D),   r = fy/(1-fy)
    -> 4 merged DVE ops [128, 8x256] (2 bcast mults + 2 adds); the
    s00/s01 scales ride the PE transposes as diagonal rhs matrices
    (PSUM-accumulated pairs), built by 2 more merged DVE ops.
  - ACT evicts PSUM->SBUF bf16 cols; 18-K-tile GEMM with 1024-col
    moving operands -> y[256, band*128]
  - GroupNorm partials via ACT accumulators; 32x2 AllReduce across the
    4 band-cores of the image; fused scale/bias/ReLU on ACT.
Conv bias cancels inside the GN centering; gamma/beta applied in the
final fused activation.
"""

import numpy as np

USE_DIAG = True               # ratio-fold + diag transposes; False = 7-op fold

# ---- problem constants (hardcoded; kernel.py must be self-contained) ----
N, C, H, W = 2, 256, 128, 128
KK = 3
GROUPS, EPS = 32, 1e-5
PADC = 2                      # zero-pad margin on each side
HP = WP = H + 2 * PADC        # 132
TOK = HP * WP                 # 17424 tokens (one per padded pixel)
BAND = 32                     # output rows per core
NB = 8                        # 512-px blocks per core (BAND*128 / 512)
NBP = 4                       # 1024-px block-pairs per core
NCORES = 8
CLAMP_LO, CLAMP_HI = 0.5, 130.4999
QUAD_K0 = 9                   # taps >= this use the 4-diag PE fold (no DVE fold)
NPIX_G = 8 * H * W            # elements per group per image

_PROG_CACHE = {}


def _build_program(n_cores):
    import concourse.bass as bass
    import concourse.tile as tile
    from concourse import bacc, mybir
    from contextlib import ExitStack

    F32 = mybir.dt.float32
    BF16 = mybir.dt.bfloat16
    I16 = mybir.dt.int16
    A = mybir.AluOpType
    AF = mybir.ActivationFunctionType

    nc = bacc.Bacc(
        "TRN2", target_bir_lowering=False, debug=False, num_devices=n_cores,
        num_swdge_queues=4,
    )

    ximg = nc.dram_tensor("ximg", [TOK + 1, 4 * C], BF16, kind="ExternalInput")
    # position tables come PRE-ADDED from the host (off + base) in f32:
    # same bytes as the old split off/add pair in bf16, one less DVE op
    # per pipeline stage, and no bf16 integral-position hazard
    pyi_d = nc.dram_tensor("pyi", [128, 2304], F32, kind="ExternalInput")
    pxi_d = nc.dram_tensor("pxi", [128, 2304], F32, kind="ExternalInput")
    # w-pipeline inputs come value-DUPLICATED in column pairs: [128, 576]
    pyw_d = nc.dram_tensor("pyw", [128, 576], F32, kind="ExternalInput")
    pxw_d = nc.dram_tensor("pxw", [128, 576], F32, kind="ExternalInput")
    # wt pre-transposed on host to [128, 18*256] so the load is one
    # contiguous DMA (the old [2304,256] AP emitted 2304 512B descriptors
    # that jammed the DMA engines for ~20us at startup)
    wt_d = nc.dram_tensor("wt", [128, 18 * 256], BF16, kind="ExternalInput")
    ident_d = nc.dram_tensor("ident", [128, 128], BF16, kind="ExternalInput")
    # gsel/gam/bet packed into one [128, 20] f32 tensor (fewer tiny DMAs)
    gnb_d = nc.dram_tensor("gnb", [128, 20], F32, kind="ExternalInput")
    gselT_d = nc.dram_tensor("gselT", [16, 128], F32, kind="ExternalInput")
    yout_d = nc.dram_tensor("yout", [128, 2 * NB * 512], F32, kind="ExternalOutput")
    ccin = nc.dram_tensor("ccin", [16, 4], F32)
    ccout = nc.dram_tensor("ccout", [16, 4], F32)

    with tile.TileContext(nc) as tc, ExitStack() as ctx:
        const = ctx.enter_context(tc.tile_pool(name="const", bufs=1))
        persist = ctx.enter_context(tc.tile_pool(name="persist", bufs=1))

        # ---- constants ----
        wt_sb = const.tile([128, 18, 256], BF16)
        nc.sync.dma_start(wt_sb, wt_d.ap())
        ident = const.tile([128, 128], BF16)
        nc.sync.dma_start(ident, ident_d.ap())
        gnb = const.tile([128, 20], F32)
        nc.sync.dma_start(gnb, gnb_d.ap())
        gsel = gnb[:, 0:16]
        gam = gnb[:, 16:18]
        bet = gnb[:, 18:20]
        gselT = const.tile([16, 128], F32)
        nc.sync.dma_start(gselT, gselT_d.ap())
        # warmup, off the critical path: preload ACT function tables that
        # would otherwise lazy-load inside the tail (Relu/Square)
        wrm = const.tile([128, 1], F32)
        nc.vector.memset(wrm, 0.0)
        nc.scalar.activation(out=wrm, in_=wrm, func=AF.Relu)
        nc.scalar.activation(out=wrm, in_=wrm, func=AF.Square)
        nc.scalar.activation(out=wrm, in_=wrm, func=AF.Sqrt)

        # ---- chunked pipeline outputs (weights stored duplicated).
        # Two wide chunks (taps 0-3, 4-8): wide ops amortize the per-op
        # dependency latency; separate tiles per chunk keep early taps'
        # gathers unblocked as soon as their chunk lands.
        CHUNKS = [(0, 2), (2, 5), (5, 9)]
        idx16 = [persist.tile([128, (hi - lo) * 256], I16, name=f"idx{lo}")
                 for lo, hi in CHUNKS]
        s00 = [persist.tile([128, (hi - lo) * 64], BF16, name=f"s00_{lo}")
               for lo, hi in CHUNKS]
        s01 = [persist.tile([128, (hi - lo) * 64], BF16, name=f"s01_{lo}")
               for lo, hi in CHUNKS]
        rd = [persist.tile([128, (hi - lo) * 64], BF16, name=f"rd{lo}")
              for lo, hi in CHUNKS]
        s10 = [persist.tile([128, (hi - lo) * 64], BF16, name=f"s10_{lo}")
               for lo, hi in CHUNKS]
        s11 = [persist.tile([128, (hi - lo) * 64], BF16, name=f"s11_{lo}")
               for lo, hi in CHUNKS]

        # main-loop pools created BEFORE the scoped preload pool so gathers
        # and folds don't anti-depend on the prologue draining; the preload
        # pool's SBUF is instead reused by the late pools (colsb/ypool/...)
        gpool = ctx.enter_context(tc.tile_pool(name="gpool", bufs=5))
        fpool = ctx.enter_context(tc.tile_pool(name="fpool", bufs=2))

        # scoped preload pool: all table inputs land via 8 big DMAs, then
        # the pool closes and its SBUF is reused by the late pools
        pipe_cm = tc.tile_pool(name="pipe", bufs=2)
        pipe = pipe_cm.__enter__()
        pre = {}
        for nm, dt_, ncol, eng in (
                ("pyi", pyi_d, 2304, "sync"), ("pxi", pxi_d, 2304, "sync"),
                ("pyw", pyw_d, 576, "gp"), ("pxw", pxw_d, 576, "gp")):
            t = pipe.tile([128, ncol], F32, tag=f"pre_{nm}", name=f"pre_{nm}",
                          bufs=1)
            if eng == "sync":
                nc.sync.dma_start(t, dt_.ap())
            else:
                nc.gpsimd.dma_start(t, dt_.ap())
            pre[nm] = t

        def pos_pipeline(o_ap, ncols, want_frac, act_floor):
            p = pipe.tile([128, ncols], F32, tag="ptmp", name="pp")
            nc.vector.tensor_scalar(out=p, in0=o_ap, scalar1=CLAMP_LO,
                                    scalar2=None, op0=A.max)
            nc.vector.tensor_scalar(out=p, in0=p, scalar1=CLAMP_HI,
                                    scalar2=None, op0=A.min)
            # floor for positive p via two fp32 adds (round-to-nearest
            # against 2^23; exact for bilinear at integer ties). The idx
            # pipelines keep these on DVE (the ACT queue is contended by
            # psT evictions); the w pipelines offload them to ACT.
            t = pipe.tile([128, ncols], F32, tag="ptmp", name="pt")
            i = pipe.tile([128, ncols], F32, tag="ipart", name="pi")
            if act_floor:
                nc.scalar.activation(out=t, in_=p, func=AF.Copy,
                                     bias=8388607.5, scale=1.0)
                nc.scalar.activation(out=i, in_=t, func=AF.Copy,
                                     bias=-8388608.0, scale=1.0)
            else:
                nc.vector.tensor_scalar(out=t, in0=p, scalar1=8388607.5,
                                        scalar2=None, op0=A.add)
                nc.vector.tensor_scalar(out=i, in0=t, scalar1=-8388608.0,
                                        scalar2=None, op0=A.add)
            if not want_frac:
                return i, None
            f = pipe.tile([128, ncols], F32, tag="frac", name="pf")
            nc.vector.tensor_tensor(out=f, in0=p, in1=i, op=A.subtract)
            return i, f

        def build_idx(c):
            lo, hi = CHUNKS[c]
            ci, ni = lo * 256, (hi - lo) * 256
            # index pipeline [128, ni]; col = k*256 + b*32 + s
            y0i, _ = pos_pipeline(pre["pyi"][:, ci:ci + ni], ni, False, False)
            x0i, _ = pos_pipeline(pre["pxi"][:, ci:ci + ni], ni, False, False)
            idxf = pipe.tile([128, ni], F32, tag="ptmp", name="idxf")
            nc.vector.scalar_tensor_tensor(out=idxf, in0=y0i, scalar=float(WP),
                                           in1=x0i, op0=A.mult, op1=A.add)
            nc.vector.tensor_copy(out=idx16[c], in_=idxf)

        def build_w(c):
            lo, hi = CHUNKS[c]
            cw, nw = lo * 64, (hi - lo) * 64
            # weight pipeline [128, nw] duplicated; col = 2*(b*4+j)+e
            _, fyw = pos_pipeline(pre["pyw"][:, cw:cw + nw], nw, True, True)
            _, fxw = pos_pipeline(pre["pxw"][:, cw:cw + nw], nw, True, True)
            wy0 = pipe.tile([128, nw], F32, tag="ipart", name="wy0")
            nc.scalar.activation(out=wy0, in_=fyw, func=AF.Copy,
                                 bias=1.0, scale=-1.0)
            # keep wy0 away from 0: bf16 offsets can make p exactly integral
            # (fy==1.0), where rd=fy/wy0 would be inf and s00*(A+rd*C) NaN.
            # Clamping wy0 (used consistently in s00/s01 AND the ratio)
            # perturbs those samples by <=2^-9 instead.
            nc.vector.tensor_scalar(out=wy0, in0=wy0, scalar1=0.001953125,
                                    scalar2=None, op0=A.max)
            wx0 = pipe.tile([128, nw], F32, tag="ipart", name="wx0")
            nc.scalar.activation(out=wx0, in_=fxw, func=AF.Copy,
                                 bias=1.0, scale=-1.0)
            nc.vector.tensor_tensor(out=s00[c], in0=wy0, in1=wx0, op=A.mult)
            nc.vector.tensor_tensor(out=s01[c], in0=wy0, in1=fxw, op=A.mult)
            if hi > QUAD_K0:
                nc.vector.tensor_tensor(out=s10[c], in0=fyw, in1=wx0,
                                        op=A.mult)
                nc.vector.tensor_tensor(out=s11[c], in0=fyw, in1=fxw,
                                        op=A.mult)
            if lo < QUAD_K0:
                wyr = pipe.tile([128, nw], F32, tag="frac", name="wyr")
                nc.vector.reciprocal(out=wyr, in_=wy0)
                nc.vector.tensor_tensor(out=rd[c], in0=fyw, in1=wyr,
                                        op=A.mult)

        # idx chunks gate the gathers, w chunks gate the folds (and thus
        # gather-buffer recycling); interleave so tap-0 folds start early
        build_idx(0)
        build_idx(1)
        build_w(0)
        build_idx(2)
        build_w(1)
        build_w(2)
        pipe_cm.__exit__(None, None, None)

        colsb = ctx.enter_context(tc.tile_pool(name="colsb", bufs=1))
        sq_p = ctx.enter_context(tc.tile_pool(name="sq", bufs=1))
        ypool = ctx.enter_context(tc.tile_pool(name="ypool", bufs=1))
        stat = ctx.enter_context(tc.tile_pool(name="stat", bufs=1))
        pcols = ctx.enter_context(tc.tile_pool(name="pcols", bufs=3, space="PSUM"))
        pgemm = ctx.enter_context(tc.tile_pool(name="pgemm", bufs=4, space="PSUM"))
        pstat = ctx.enter_context(tc.tile_pool(name="pstat", bufs=1, space="PSUM"))

        # each token = all 4 bilinear corners (2 rows x 2 cols x 256 ch) bf16
        gsrc = bass.AP(tensor=ximg, offset=0, ap=[[1024, TOK], [1, 1024]])

        ysb = ypool.tile([128, 2, NB, 512], F32)
        sacc = stat.tile([128, 2, NB], F32)
        qacc = stat.tile([128, 2, NB], F32)

        def dupb(t, c0, n0, inner):
            # dup'd tile slice: value j at cols (2j, 2j+1); bcast x inner
            return bass.AP(tensor=t.tensor, offset=t.offset + 2 * c0,
                           ap=[t.ap[0], [2, n0], [0, inner // 2], [1, 2]])

        identb = bass.AP(tensor=ident.tensor, offset=ident.offset,
                         ap=[ident.ap[0], [0, 8], [1, 128]])

        # ---- main loop over 1024-px block-pairs ----
        for bp in range(NBP):
            colsA = colsb.tile([128, 10, 1024], BF16, tag="colsA", name="colsA")
            colsB = colsb.tile([128, 8, 1024], BF16, tag="colsB", name="colsB")
            pg3 = {}
            for k in range(9):
                c = 0 if k < 2 else (1 if k < 5 else 2)
                kl = k - CHUNKS[c][0]
                icol = (kl * 8 + 2 * bp) * 32
                g0 = gpool.tile([128, 8, 1024], BF16, tag="g0", name="g0")
                if bp == 0 and k < 4:
                    # ramp: split the first gathers into 512-idx halves on
                    # distinct queues so all 4 Q7 pairs engage immediately
                    for hf in range(2):
                        nc.gpsimd.dma_gather(
                            out_ap=g0[:, 4 * hf:4 * (hf + 1), :], in_ap=gsrc,
                            idxs_ap=idx16[c][:, icol + 32 * hf:icol + 32 * (hf + 1)],
                            num_idxs=512, num_idxs_reg=512,
                            elem_size=1024, elem_step=1024,
                            queue_num=(2 * k + hf) % 4,
                        )
                else:
                    nc.gpsimd.dma_gather(
                        out_ap=g0, in_ap=gsrc,
                        idxs_ap=idx16[c][:, icol:icol + 64],
                        num_idxs=1024, num_idxs_reg=1024,
                        elem_size=1024, elem_step=1024,
                        queue_num=(bp * 9 + k) % 4,
                    )
                w0 = kl * 32 + 8 * bp
                # corner views [128, 8, 256]: A=y0x0 B=y0x1 C=y1x0 D=y1x1
                cn = [bass.AP(tensor=g0.tensor, offset=g0.offset + c * 256,
                              ap=[g0.ap[0], [1024, 8], [1, 256]])
                      for c in range(4)]
                if k >= QUAD_K0 and bp == NBP - 1:
                    # 4-diag PE fold: psum[ch,px] = A^T@d00 + B^T@d01
                    #                             + C^T@d10 + D^T@d11
                    # (no DVE fold ops at all; corners feed PE directly)
                    d00 = fpool.tile([128, 8, 128], BF16, tag="d00", name="d00")
                    nc.vector.tensor_tensor(out=d00, in0=identb,
                                            in1=dupb(s00[c], w0, 8, 128), op=A.mult)
                    d01 = fpool.tile([128, 8, 128], BF16, tag="d01", name="d01")
                    nc.vector.tensor_tensor(out=d01, in0=identb,
                                            in1=dupb(s01[c], w0, 8, 128), op=A.mult)
                    d10 = fpool.tile([128, 8, 128], BF16, tag="d10", name="d10")
                    nc.vector.tensor_tensor(out=d10, in0=identb,
                                            in1=dupb(s10[c], w0, 8, 128), op=A.mult)
                    d11 = fpool.tile([128, 8, 128], BF16, tag="d11", name="d11")
                    nc.vector.tensor_tensor(out=d11, in0=identb,
                                            in1=dupb(s11[c], w0, 8, 128), op=A.mult)
                    dsets = [d00, d01, d10, d11]
                    corder = [0, 1, 2, 3]
                    for h in range(2):
                        for half in range(2):
                            psT = pcols.tile([128, 512], F32, tag="psT",
                                             name="psT")
                            for q in range(4):
                                jj = half * 4 + q
                                sl = psT[:, q * 128:(q + 1) * 128]
                                for ci4 in range(4):
                                    nc.tensor.matmul(
                                        out=sl,
                                        lhsT=cn[corder[ci4]][:, jj,
                                                             h * 128:(h + 1) * 128],
                                        rhs=dsets[ci4][:, jj, :],
                                        start=(ci4 == 0), stop=(ci4 == 3))
                            kt_ = 2 * k + h
                            ct, kc = ((colsA, kt_) if kt_ < 10 else
                                      (colsB, kt_ - 10))
                            nc.scalar.copy(
                                out=ct[:, kc, half * 512:(half + 1) * 512],
                                in_=psT)
                elif USE_DIAG:
                    # a0 = A + r*C ; a1 = B + r*D   (r = fy/(1-fy))
                    m0 = fpool.tile([128, 8, 256], BF16, tag="m0", name="m0")
                    nc.vector.tensor_tensor(out=m0, in0=cn[2],
                                            in1=dupb(rd[c], w0, 8, 256), op=A.mult)
                    a0 = fpool.tile([128, 8, 256], BF16, tag="a0", name="a0")
                    nc.vector.tensor_tensor(out=a0, in0=cn[0], in1=m0, op=A.add)
                    m1 = fpool.tile([128, 8, 256], BF16, tag="m0", name="m1")
                    nc.vector.tensor_tensor(out=m1, in0=cn[3],
                                            in1=dupb(rd[c], w0, 8, 256), op=A.mult)
                    a1 = fpool.tile([128, 8, 256], BF16, tag="a1", name="a1")
                    nc.vector.tensor_tensor(out=a1, in0=cn[1], in1=m1, op=A.add)
                    # diag sets: D8x[p, jj, f] = ident[p, f] * sxx[p, w0+jj]
                    d00 = fpool.tile([128, 8, 128], BF16, tag="d00", name="d00")
                    nc.vector.tensor_tensor(out=d00, in0=identb,
                                            in1=dupb(s00[c], w0, 8, 128), op=A.mult)
                    d01 = fpool.tile([128, 8, 128], BF16, tag="d01", name="d01")
                    nc.vector.tensor_tensor(out=d01, in0=identb,
                                            in1=dupb(s01[c], w0, 8, 128), op=A.mult)
                    # transpose+scale: psum[ch,px] = a0^T@diag(s00)+a1^T@diag(s01)
                    for h in range(2):
                        for half in range(2):
                            psT = pcols.tile([128, 512], F32, tag="psT",
                                             name="psT")
                            for q in range(4):
                                jj = half * 4 + q
                                sl = psT[:, q * 128:(q + 1) * 128]
                                nc.tensor.matmul(
                                    out=sl,
                                    lhsT=a0[:, jj, h * 128:(h + 1) * 128],
                                    rhs=d00[:, jj, :], start=True, stop=False)
                                nc.tensor.matmul(
                                    out=sl,
                                    lhsT=a1[:, jj, h * 128:(h + 1) * 128],
                                    rhs=d01[:, jj, :], start=False, stop=True)
                            kt_ = 2 * k + h
                            ct, kc = ((colsA, kt_) if kt_ < 10 else
                                      (colsB, kt_ - 10))
                            nc.scalar.copy(
                                out=ct[:, kc, half * 512:(half + 1) * 512],
                                in_=psT)
                else:
                    ta = fpool.tile([128, 8, 256], BF16, tag="ta", name="ta")
                    nc.vector.tensor_tensor(out=ta, in0=cn[0],
                                            in1=dupb(s00[k], w0, 8, 256), op=A.mult)
                    tb = fpool.tile([128, 8, 256], BF16, tag="tb", name="tb")
                    nc.vector.tensor_tensor(out=tb, in0=cn[2],
                                            in1=dupb(s10[k], w0, 8, 256), op=A.mult)
                    u0 = fpool.tile([128, 8, 256], BF16, tag="u0", name="u0")
                    nc.vector.tensor_tensor(out=u0, in0=ta, in1=tb, op=A.add)
                    tc_ = fpool.tile([128, 8, 256], BF16, tag="ta", name="tc2")
                    nc.vector.tensor_tensor(out=tc_, in0=cn[1],
                                            in1=dupb(s01[k], w0, 8, 256), op=A.mult)
                    td = fpool.tile([128, 8, 256], BF16, tag="tb", name="td")
                    nc.vector.tensor_tensor(out=td, in0=cn[3],
                                            in1=dupb(s11[k], w0, 8, 256), op=A.mult)
                    u1 = fpool.tile([128, 8, 256], BF16, tag="u1", name="u1")
                    nc.vector.tensor_tensor(out=u1, in0=tc_, in1=td, op=A.add)
                    smp = fpool.tile([128, 8, 256], BF16, tag="smp", name="smp")
                    nc.vector.tensor_tensor(out=smp, in0=u0, in1=u1, op=A.add)
                    for h in range(2):
                        for half in range(2):
                            psT = pcols.tile([128, 512], F32, tag="psT",
                                             name="psT")
                            for q in range(4):
                                jj = half * 4 + q
                                nc.tensor.matmul(
                                    out=psT[:, q * 128:(q + 1) * 128],
                                    lhsT=smp[:, jj, h * 128:(h + 1) * 128],
                                    rhs=ident, start=True, stop=True)
                            kt_ = 2 * k + h
                            ct, kc = ((colsA, kt_) if kt_ < 10 else
                                      (colsB, kt_ - 10))
                            nc.scalar.copy(
                                out=ct[:, kc, half * 512:(half + 1) * 512],
                                in_=psT)
                if k == 4:
                    for m in range(2):
                        for half in range(2):
                            pg = pgemm.tile([128, 512], F32, tag="pg",
                                            name=f"pgp{m}{half}")
                            pg3[(m, half)] = pg
                            for kt in range(10):
                                nc.tensor.matmul(
                                    out=pg,
                                    lhsT=wt_sb[:, kt, m * 128:(m + 1) * 128],
                                    rhs=colsA[:, kt,
                                              half * 512:(half + 1) * 512],
                                    start=(kt == 0), stop=False,
                                    skip_group_check=True)
            for m in range(2):
                for half in range(2):
                    b = 2 * bp + half
                    pg = pg3[(m, half)]
                    for kt in range(10, 18):
                        nc.tensor.matmul(
                            out=pg, lhsT=wt_sb[:, kt, m * 128:(m + 1) * 128],
                            rhs=colsB[:, kt - 10, half * 512:(half + 1) * 512],
                            start=False, stop=(kt == 17),
                            skip_group_check=True)
                    nc.scalar.activation(
                        out=ysb[:, m, b, :], in_=pg, func=AF.Copy,
                        accum_out=sacc[:, m, b:b + 1])
                    sq = sq_p.tile([128, 512], F32, tag="sq", name="sq")
                    nc.scalar.activation(
                        out=sq, in_=pg, func=AF.Square,
                        accum_out=qacc[:, m, b:b + 1])

        # ---- GroupNorm stats + AllReduce ----
        stot = stat.tile([128, 2], F32)
        nc.vector.tensor_reduce(out=stot, in_=sacc, axis=mybir.AxisListType.X,
                                op=A.add)
        qtot = stat.tile([128, 2], F32)
        nc.vector.tensor_reduce(out=qtot, in_=qacc, axis=mybir.AxisListType.X,
                                op=A.add)
        st4 = stat.tile([128, 4], F32)
        nc.vector.tensor_copy(out=st4[:, 0:2], in_=stot)
        nc.vector.tensor_copy(out=st4[:, 2:4], in_=qtot)
        psg = pstat.tile([16, 4], F32, tag="pst", name="psg")
        nc.tensor.matmul(out=psg, lhsT=gsel, rhs=st4, start=True, stop=True)
        cc_sb = stat.tile([16, 4], F32)
        nc.vector.tensor_copy(out=cc_sb, in_=psg)
        nc.sync.dma_start(ccin.ap(), cc_sb)
        if n_cores == 8:
            nc.gpsimd.collective_compute(
                "AllReduce", A.add,
                replica_groups=[[0, 1, 2, 3], [4, 5, 6, 7]],
                ins=[ccin.ap()], outs=[ccout.ap()],
            )
            ccr = stat.tile([16, 4], F32)
            nc.sync.dma_start(ccr, ccout.ap())
        else:
            ccr = stat.tile([16, 4], F32)
            nc.sync.dma_start(ccr, ccin.ap())

        # mean = s/Npix ; var = q/Npix - mean^2 ; rstd = rsqrt(var + eps)
        mr = stat.tile([16, 4], F32)
        nc.vector.tensor_scalar(out=mr[:, 0:2], in0=ccr[:, 0:2],
                                scalar1=1.0 / NPIX_G, scalar2=None, op0=A.mult)
        varq = stat.tile([16, 2], F32)
        nc.vector.tensor_scalar(out=varq, in0=ccr[:, 2:4],
                                scalar1=1.0 / NPIX_G, scalar2=None, op0=A.mult)
        msq = stat.tile([16, 2], F32)
        nc.vector.tensor_tensor(out=msq, in0=mr[:, 0:2], in1=mr[:, 0:2],
                                op=A.mult)
        nc.vector.tensor_tensor(out=varq, in0=varq, in1=msq, op=A.subtract)
        epst = stat.tile([16, 1], F32)
        nc.vector.memset(epst, EPS)
        nc.scalar.activation(out=varq, in_=varq,
                             func=AF.Sqrt, bias=epst, scale=1.0)
        nc.vector.reciprocal(out=mr[:, 2:4], in_=varq)
        # broadcast 16 group stats to 128 partitions on PE (one-hot gselT)
        # instead of the old HBM round-trip (saves ~8us of tail latency)
        pmb = pstat.tile([128, 4], F32, tag="pst", name="pmb")
        nc.tensor.matmul(out=pmb, lhsT=gselT, rhs=mr, start=True, stop=True)
        mrc = stat.tile([128, 4], F32)
        nc.vector.tensor_copy(out=mrc, in_=pmb)
        scale_c = stat.tile([128, 2], F32)
        nc.vector.tensor_tensor(out=scale_c, in0=gam, in1=mrc[:, 2:4], op=A.mult)
        shift_c = stat.tile([128, 2], F32)
        nc.vector.tensor_tensor(out=shift_c, in0=mrc[:, 0:2], in1=scale_c,
                                op=A.mult)
        nc.vector.tensor_tensor(out=shift_c, in0=bet, in1=shift_c, op=A.subtract)

        # ---- fused normalize + relu + store ----
        yv = yout_d.ap().rearrange("p (m f) -> p m f", m=2)
        for m in range(2):
            for q in range(4):
                hv = bass.AP(
                    tensor=ysb.tensor,
                    offset=ysb.offset + (m * NB + q * 2) * 512,
                    ap=[ysb.ap[0], [1, 1024]])
                nc.scalar.activation(
                    out=hv, in_=hv, func=AF.Relu,
                    scale=scale_c[:, m:m + 1], bias=shift_c[:, m:m + 1],
                )
                nc.sync.dma_start(yv[:, m, q * 1024:(q + 1) * 1024], hv)

    nc.compile()
    return nc


def _get_program(n_cores=NCORES):
    if n_cores not in _PROG_CACHE:
        _PROG_CACHE[n_cores] = _build_program(n_cores)
    return _PROG_CACHE[n_cores]


def _host_prep(x, offset, weight, bias, gamma, beta):
    """Build the 8 per-core input maps (layout prep only; all math on device)."""
    x = np.ascontiguousarray(x, np.float32)
    offset = np.ascontiguousarray(offset, np.float32)
    weight = np.ascontiguousarray(weight, np.float32)
    gamma = np.ascontiguousarray(gamma, np.float32)
    beta = np.ascontiguousarray(beta, np.float32)

    import ml_dtypes
    # 4-corner token layout: token (y, x) = [ (y,x), (y,x+1), (y+1,x),
    # (y+1,x+1) ] x 256 ch, so one dma_gather descriptor fetches a full
    # bilinear footprint. Built from a zero-extended padded image.
    xp = np.pad(x, ((0, 0), (0, 0), (PADC, PADC + 1), (PADC, PADC + 1)))
    xcl = np.transpose(xp, (0, 2, 3, 1)).astype(ml_dtypes.bfloat16)  # [N,133,133,C]
    ximg = np.empty((N, TOK + 1, 4, C), ml_dtypes.bfloat16)
    a = xcl[:, :HP, :WP]
    ximg[:, :TOK, 0] = a.reshape(N, TOK, C)
    ximg[:, :TOK, 1] = xcl[:, :HP, 1:WP + 1].reshape(N, TOK, C)
    ximg[:, :TOK, 2] = xcl[:, 1:HP + 1, :WP].reshape(N, TOK, C)
    ximg[:, :TOK, 3] = xcl[:, 1:HP + 1, 1:WP + 1].reshape(N, TOK, C)
    ximg[:, TOK:] = 0
    ximg = np.ascontiguousarray(ximg.reshape(N, TOK + 1, 4 * C))

    # wt stored pre-transposed: wt[p, kt, c] = W[oc=kt%2*128+... ] layout so
    # the device load is one contiguous [128, 4608] DMA
    wt = np.empty((128, 18, 256), np.float32)
    for kt in range(18):
        tap, half = kt // 2, kt % 2
        ki, kj = tap // 3, tap % 3
        wt[:, kt, :] = weight[:, half * 128:(half + 1) * 128, ki, kj].T
    wt = np.ascontiguousarray(wt.reshape(128, 18 * 256)).astype(ml_dtypes.bfloat16)
    ident = np.eye(128, dtype=np.float32).astype(ml_dtypes.bfloat16)
    gsel = np.zeros((128, 16), np.float32)
    gsel[np.arange(128), np.arange(128) // 8] = 1.0
    gam2 = gamma.reshape(2, 128).T.copy()
    bet2 = beta.reshape(2, 128).T.copy()
    gnb = np.ascontiguousarray(
        np.concatenate([gsel, gam2, bet2], axis=1), np.float32)
    gselT = np.ascontiguousarray(gsel.T, np.float32)

    # index-pipeline layouts (replication of raw offset bytes + structural consts)
    p = np.arange(128)[:, None]
    ci = np.arange(2304)[None, :]
    k_i = ci // 256
    b_i = (ci // 32) % 8
    s_i = ci % 32
    i_i = s_i * 16 + (p % 16)
    hl_i = 4 * b_i + s_i // 8
    w_i = 16 * (s_i % 8) + (p % 16)
    cw = np.arange(288)[None, :]
    k_w = cw // 32
    b_w = (cw // 4) % 8
    j_w = cw % 4
    hl_w = 4 * b_w + j_w
    w_w = np.broadcast_to(p, (128, 288))

    def dup(a):
        return np.repeat(np.asarray(a, np.float32), 2, axis=1)

    in_maps = []
    for core in range(NCORES):
        n_img, q = core // 4, core % 4
        h0 = BAND * q
        offb = offset[n_img, :, h0:h0 + BAND, :]
        oyi = offb[2 * k_i, hl_i, w_i]
        oxi = offb[2 * k_i + 1, hl_i, w_i]
        oyw = offb[2 * k_w, hl_w, w_w]
        oxw = offb[2 * k_w + 1, hl_w, w_w]
        addyi = (k_i // 3 - 1 + h0 + hl_i + PADC) + 0.0 * p
        addxi = (k_i % 3 - 1 + w_i + PADC) + 0.0 * p
        addyw = (k_w // 3 - 1 + h0 + hl_w + PADC) + 0.0 * w_w
        addxw = (k_w % 3 - 1 + w_w + PADC) + 0.0 * w_w
        in_maps.append({
            "ximg": ximg[n_img],
            "pyi": np.ascontiguousarray(oyi + addyi, np.float32),
            "pxi": np.ascontiguousarray(oxi + addxi, np.float32),
            "pyw": np.ascontiguousarray(dup(oyw + addyw)),
            "pxw": np.ascontiguousarray(dup(oxw + addxw)),
            "wt": wt,
            "ident": ident,
            "gnb": gnb,
            "gselT": gselT,
        })
    return in_maps


def _assemble(results):
    out = np.empty((N, C, H, W), np.float32)
    for core, res in enumerate(results):
        n_img, q = core // 4, core % 4
        arr = res["yout"].reshape(128, 2, NB, 4, 128)
        band = np.transpose(arr, (1, 0, 2, 3, 4)).reshape(C, BAND, W)
        out[n_img, :, BAND * q:BAND * (q + 1), :] = band
    return out


def run(inputs, trace=False, trace_kwargs=None):
    from concourse.bass_utils import run_bass_kernel_spmd
    nc = _get_program(NCORES)
    in_maps = _host_prep(**inputs)
    r = run_bass_kernel_spmd(
        nc, in_maps, core_ids=list(range(NCORES)),
        trace=trace, **(trace_kwargs or {}),
    )
    return _assemble(r.results), r


def kernel(**inputs) -> np.ndarray:
    out, _ = run(inputs, trace=False)
    return out

